# revision 1
# baseline (speedup 1.0000x reference)
"""ConvAConnect Trainium2 kernel.

Per-sample noisy conv: Z[b] = conv2d(X[b], W * Werr[b], VALID) + bias * Berr[b].

Strategy: data-parallel over batch across 8 NeuronCores (8 samples each).
Per core, the conv is lowered to 9 shifted matmuls (one per 3x3 tap)
accumulating in PSUM:
  out[(ho,wo), cout] += X[(ho+kh, wo+kw), cin] @ (W*Werr)[kh,kw,cin,cout]
with Cin=128 exactly the PE contraction dim. X is pre-transposed on the
host to [Cin, H*W] so both matmul operands have Cin on partitions and all
DMAs are contiguous. Matmuls run in float32r (fp32 operands, FP22
multiply, full PE rate at moving-dim >= 256), accumulate fp32 in PSUM.
Output chunks are 2 output rows in 64-wide row coordinates (M=128
partitions with 2 dead columns per row, N=256); the stationary for each
tap is then a single contiguous X slab (walrus requires one free dim),
and chunk stores are single full-partition DMAs whose dead columns the
host strips. The per-sample bias (bias * Berr[b]) is added during the
PSUM->SBUF move by the DVE.
"""

import numpy as np

B, H, Wd, CIN, COUT, KH, KW = 64, 64, 64, 128, 256, 3, 3
HO, WO = H - KH + 1, Wd - KW + 1  # 62, 62
NCORES = 8
S = B // NCORES  # samples per core
ROWS_PER_CHUNK = 2
NCHUNK = HO // ROWS_PER_CHUNK  # 31
M = ROWS_PER_CHUNK * WO  # 124

PAD = 64  # X tile free-dim pad: last chunk's kh=2/kw>0 taps read past H*W

TRACE = False  # set by test harness to capture an NTFF profile
LAST_RESULTS = None  # BassKernelResults of the most recent run (for profiling)

_prog_cache = None


def _build_program():
    import concourse.mybir as mybir
    from concourse import bacc
    from concourse.tile import TileContext
    from concourse.tile_rust import add_dep_helper

    f32 = mybir.dt.float32
    f32r = mybir.dt.float32r

    # Bacc (not plain Bass): its compile() runs generate_event_semaphores,
    # which splits multi-sem waits into EventSemaphore chains — walrus
    # codegen rejects instructions carrying more than ~2 sync waits.
    nc = bacc.Bacc()

    # X_t is declared float32r (same bytes as f32): walrus requires data
    # consumed by an f32r matmul to be produced as f32r along the whole chain.
    # The free dim carries a host-zeroed PAD so the last chunk's kh=2 taps
    # can read one full 128-wide stationary without going out of bounds.
    X_t = nc.declare_dram_parameter(
        "X_t", [S, CIN, H * Wd + PAD], f32r, isOutput=False
    )
    # W and Werr are host-pre-arranged to the SBUF layout [cin, (tap cout)]
    # so their loads are single contiguous-per-partition 2D descriptors
    W_p = nc.declare_dram_parameter("W", [CIN, KH * KW * COUT], f32, isOutput=False)
    bias_p = nc.declare_dram_parameter("bias", [COUT], f32, isOutput=False)
    Werr_p = nc.declare_dram_parameter(
        "Werr", [S, CIN, KH * KW * COUT], f32, isOutput=False
    )
    Berr_p = nc.declare_dram_parameter("Berr", [S, COUT], f32, isOutput=False)
    # output rows are stored 64 wide (2 dead columns) so each chunk is one
    # full-partition DMA; the host strips the padding
    OUT = nc.declare_dram_parameter("OUT", [S, HO * Wd, COUT], f32, isOutput=True)

    TAPF = KH * KW * COUT  # 2304 free elems: tap t occupies cols [t*COUT, (t+1)*COUT)

    with TileContext(nc) as tc:
        with (
            tc.tile_pool(name="const", bufs=1) as cpool,
            tc.tile_pool(name="xp", bufs=2) as xpool,
            tc.tile_pool(name="wep", bufs=2) as wepool,
            tc.tile_pool(name="mwp", bufs=2) as mwpool,
            tc.tile_pool(name="bbp", bufs=2) as bbpool,
            tc.tile_pool(name="outp", bufs=8) as opool,
            tc.tile_pool(name="ps", bufs=8, space="PSUM") as pspool,
        ):
            HEAD = 3 * COUT  # taps 0-2: the startup-critical slice
            # W taps, resident all run: [cin, (t cout)]; head first so the
            # first memW mul only waits on 0.4MiB of W
            W_sb = cpool.tile([CIN, TAPF], f32)
            nc.sync.dma_start(out=W_sb[:, :HEAD], in_=W_p[:, :HEAD])
            nc.sync.dma_start(out=W_sb[:, HEAD:], in_=W_p[:, HEAD:])
            # bias broadcast to all partitions: [128, COUT]
            bias_bc = cpool.tile([128, COUT], f32)
            nc.gpsimd.dma_start(out=bias_bc, in_=bias_p[:].partition_broadcast(128))

            # PE pre-warm: dummy matmuls with no DMA dependency run during
            # the startup DMA window so the HAM clock gate reaches 2.4GHz
            # before the first real matmul. 20 dummies end ~2us before the
            # real stream starts — short enough that HAM stays warm, and
            # they can't push the real stream later on fast-data runs.
            warm = cpool.tile([128, 384], f32)
            nc.vector.memset(warm, 1.0)
            ps_warm = pspool.tile([128, COUT], f32, tag="ps")
            NWARM = 20
            for i in range(NWARM):
                nc.tensor.matmul(
                    ps_warm[:],
                    warm[:, :128],
                    warm[:, 128:],
                    start=(i == 0),
                    stop=(i == NWARM - 1),
                )


            XP = 16 * Wd
            s0_last_werr = None  # s0's final Werr slice DMA
            for s in range(S):
                # X piece 0 first: the first chunks only need the top rows
                X_sb = xpool.tile([CIN, H * Wd + PAD], f32r)
                xp0_dma = nc.sync.dma_start(out=X_sb[:, :XP], in_=X_t[s, :, :XP])
                if s == 1 and s0_last_werr is not None:
                    # hold the s1 prefetch until s0's Werr has fully landed:
                    # the DMA fabric round-robins packets across outstanding
                    # transfers, so an early prefetch starves s0's
                    # startup-critical loads (s1 still has ~29us of slack)
                    add_dep_helper(
                        xp0_dma.ins,
                        s0_last_werr.ins,
                        sync=True,
                        reason="s1 prefetch yields bandwidth to s0 startup",
                    )

                # Werr head (taps 0-2) gates the first matmuls; the tail
                # (taps 3-8) streams in while the head's chunks compute.
                # DVE muls in 3 groups interleave with the per-chunk adds.
                Werr_sb = wepool.tile([CIN, TAPF], f32)
                memW = mwpool.tile([CIN, TAPF], f32r)
                nc.sync.dma_start(out=Werr_sb[:, :HEAD], in_=Werr_p[s, :, :HEAD])
                nc.vector.tensor_mul(
                    memW[:, :HEAD], W_sb[:, :HEAD], Werr_sb[:, :HEAD]
                )
                for g in range(1, 3):
                    lo, hi = g * 3 * COUT, (g + 1) * 3 * COUT
                    wdma = nc.sync.dma_start(
                        out=Werr_sb[:, lo:hi], in_=Werr_p[s, :, lo:hi]
                    )
                    nc.vector.tensor_mul(
                        memW[:, lo:hi], W_sb[:, lo:hi], Werr_sb[:, lo:hi]
                    )
                    if s == 0:
                        s0_last_werr = wdma

                # second X piece streams in behind
                nc.sync.dma_start(out=X_sb[:, XP:], in_=X_t[s, :, XP:])

                berr_bc = bbpool.tile([128, COUT], f32)
                nc.gpsimd.dma_start(
                    out=berr_bc, in_=Berr_p[s].partition_broadcast(128)
                )
                membias = bbpool.tile([128, COUT], f32)
                nc.vector.tensor_mul(membias, bias_bc, berr_bc)

                # Each chunk covers 2 output rows as 128 PSUM partitions in
                # 64-wide row coordinates: partition m = (ho - 2c)*64 + wo,
                # wo in [0,64) with wo in {62,63} dead. The tap (kh,kw)
                # stationary is then the single contiguous X slab starting at
                # (2c+kh)*64 + kw — one free dim, as walrus requires.
                for c in range(NCHUNK):
                    ps = pspool.tile([128, COUT], f32, tag="ps")
                    mm = 0
                    for kh in range(KH):
                        for kw in range(KW):
                            t = kh * KW + kw
                            base = (ROWS_PER_CHUNK * c + kh) * Wd + kw
                            lhsT = X_sb[:, base : base + 128]
                            rhs = memW[:, t * COUT : (t + 1) * COUT]  # [128, 256]
                            nc.tensor.matmul(
                                ps[:],
                                lhsT,
                                rhs,
                                start=(mm == 0),
                                stop=(mm == KH * KW - 1),
                            )
                            mm += 1
                    o_sb = opool.tile([128, COUT], f32)
                    nc.vector.tensor_add(o_sb, ps, membias)
                    # out stores ride ACT's HWDGE so SP's queue clocks (wide
                    # X/Werr loads) and these narrow stores stay independent
                    nc.scalar.dma_start(
                        out=OUT[s, 128 * c : 128 * (c + 1), :], in_=o_sb
                    )

    nc.compile()
    return nc


def _get_program():
    global _prog_cache
    if _prog_cache is None:
        _prog_cache = _build_program()
    return _prog_cache


def kernel(X, W, bias, Werr, Berr):
    global LAST_RESULTS
    from concourse.bass_utils import run_bass_kernel_spmd

    X = np.asarray(X, dtype=np.float32)
    W = np.asarray(W, dtype=np.float32)
    bias = np.asarray(bias, dtype=np.float32)
    Werr = np.asarray(Werr, dtype=np.float32)
    Berr = np.asarray(Berr, dtype=np.float32)

    # host-side layout prep (part of sharding): Cin onto partitions, zero pad
    X_t = np.zeros((B, CIN, H * Wd + PAD), np.float32)
    X_t[:, :, : H * Wd] = X.transpose(0, 3, 1, 2).reshape(B, CIN, H * Wd)
    # [kh,kw,cin,cout] -> [cin, (tap cout)] (SBUF layout, contiguous loads)
    W2 = np.ascontiguousarray(
        W.reshape(KH * KW, CIN, COUT).transpose(1, 0, 2).reshape(CIN, KH * KW * COUT)
    )
    Werr2 = np.ascontiguousarray(
        Werr.reshape(B, KH * KW, CIN, COUT)
        .transpose(0, 2, 1, 3)
        .reshape(B, CIN, KH * KW * COUT)
    )
    Berr2 = np.ascontiguousarray(Berr)

    nc = _get_program()
    in_maps = []
    for core in range(NCORES):
        sl = slice(core * S, (core + 1) * S)
        in_maps.append(
            {
                "X_t": X_t[sl],
                "W": W2,
                "bias": bias,
                "Werr": Werr2[sl],
                "Berr": Berr2[sl],
            }
        )

    res = run_bass_kernel_spmd(
        nc, in_maps, core_ids=list(range(NCORES)), trace=TRACE
    )
    LAST_RESULTS = res
    out = np.concatenate([r["OUT"] for r in res.results], axis=0)
    # rows are stored 64 wide on device; strip the 2 dead columns
    return np.ascontiguousarray(
        out.reshape(B, HO, Wd, COUT)[:, :, :WO, :]
    )

